# revision 1
# baseline (speedup 1.0000x reference)
"""FourierCrossAttention Trainium2 kernel.

Sharding: one head per NeuronCore (H=8, n_cores=8); each core processes all
B=16 batches for its head.

Math (per (b,h)):
  ftq = qT @ [cos | -sin]           (DFT, first 64 modes; fp16 hi/lo split)
  ftk = kT @ [cos | -sin]
  xy[y,x]   = sum_e ftk[e,y]*ftq[e,x]      (complex)
  A = tanh(xy)                             (complex tanh, stable form)
  v[e,x]    = sum_y ftk[e,y]*A[x,y]        (complex)
  X[o,x]    = sum_e v[e,x]*W[e,o,x]        (complex, per-head weights)
  out[o,l]  = sum_x Re(X)*cr[x,l] + Im(X)*ci[x,l]   (inverse rDFT, /(E*E))

Pipeline: two half-batches of 8 so group 1's DFT overlaps group 0's tanh.
"""
import sys
sys.path.insert(0, '/opt/trn_rl_repo')
import numpy as np
from contextlib import ExitStack

import concourse.bacc as bacc
import concourse.mybir as mybir
import concourse.tile as tile
from concourse import masks
from concourse.bass_utils import run_bass_kernel_spmd

F32 = mybir.dt.float32
F16 = mybir.dt.float16
AF = mybir.ActivationFunctionType
ALU = mybir.AluOpType

B, E, H, L = 16, 64, 8, 2048
M = 64                      # modes
NCH = 16                    # l-chunks of 128
S_W = 4096.0                # weight prescale (2^12)
S_X = 2.0 ** -18            # X downcast scale (keeps Xs out of fp16 subnormals)
S_OUT = 2.0 ** 14           # 2^-6 remainder * 2^20 fp16-range boost
S_HOST = 2.0 ** -20         # host-side unscale
S_C = 2.0 ** -11            # folded into Cinv (1/L)
# S_W * S_X * S_C * S_OUT = 2^12 * 2^-18 * 2^-11 * 2^-6 = 2^-23 = 1/(L*E*E)

INV2PI = float(1.0 / (2 * np.pi))
MAGIC = 12582912.0          # 1.5 * 2^23 round-to-int magic
C1 = 6.28125                # Cody-Waite 2pi hi (exact in 9 bits)
C2W = float(2 * np.pi - 6.28125)
HALF_PI = float(np.pi / 2)

_CACHE = {}


def _dft_consts():
    l = np.arange(L)[:, None]
    m = np.arange(M)[None, :]
    ang = 2.0 * np.pi * l * m / L
    c2 = np.concatenate([np.cos(ang), -np.sin(ang)], axis=1)   # [L, 128]
    c2hi = c2.astype(np.float16)
    c2lo = (c2 - c2hi.astype(np.float64)).astype(np.float16)
    c2hi = c2hi.reshape(128, NCH, 128)
    c2lo = c2lo.reshape(128, NCH, 128)

    x = np.arange(M)[:, None]
    lr = np.arange(L)[None, :]
    ang2 = 2.0 * np.pi * x * lr / L
    dup = np.where(np.arange(M) == 0, 1.0, 2.0)[:, None]
    cr = dup * np.cos(ang2) * (S_C * S_OUT)
    ci = -dup * np.sin(ang2) * (S_C * S_OUT)
    ci[0, :] = 0.0
    cinv = np.concatenate([cr, ci], axis=0).astype(np.float16)  # [128, L]
    return c2hi, c2lo, cinv


def _tanh_block(nc, tb, ps_re, ps_im, pack_re, pack_im, g):
    """Complex tanh of ps_re + i*ps_im ([64, 512] psum tiles); writes the
    e2 rhs packs: pack_re=[2t/D ; -Im], pack_im=[Im ; 2t/D] as fp16."""
    t = tb.tile([64, 512], F32, tag="t")
    nc.scalar.activation(t[:], ps_re[:], AF.Tanh)
    # range reduction of z = 2*Im mod 2pi (Cody-Waite)
    u1 = tb.tile([64, 512], F32, tag="u1")
    nc.vector.tensor_scalar(u1[:], ps_im[:], 2.0 * INV2PI, MAGIC,
                            ALU.mult, ALU.add)
    w1 = tb.tile([64, 512], F32, tag="w1")
    nc.vector.tensor_scalar(w1[:], u1[:], MAGIC, C1, ALU.subtract, ALU.mult)
    w2 = tb.tile([64, 512], F32, tag="w2")
    nc.vector.tensor_scalar(w2[:], u1[:], MAGIC, C2W, ALU.subtract, ALU.mult)
    y2 = tb.tile([64, 512], F32, tag="y2")
    nc.vector.tensor_scalar_mul(y2[:], ps_im[:], 2.0)
    zr0 = tb.tile([64, 512], F32, tag="zr0")
    nc.vector.tensor_sub(zr0[:], y2[:], w1[:])
    zr = tb.tile([64, 512], F32, tag="zr")
    nc.vector.tensor_sub(zr[:], zr0[:], w2[:])
    s2 = tb.tile([64, 512], F32, tag="s2")
    nc.scalar.activation(s2[:], zr[:], AF.Sin)
    sh = tb.tile([64, 512], F32, tag="sh")
    nc.scalar.activation(sh[:], zr[:], AF.Sin, scale=0.5)
    sh2 = tb.tile([64, 512], F32, tag="sh2")
    nc.vector.tensor_mul(sh2[:], sh[:], sh[:])
    c2t = tb.tile([64, 512], F32, tag="c2t")
    nc.gpsimd.tensor_scalar(c2t[:], sh2[:], -2.0, 1.0, ALU.mult, ALU.add)
    t2 = tb.tile([64, 512], F32, tag="t2")
    nc.vector.tensor_mul(t2[:], t[:], t[:])
    u = tb.tile([64, 512], F32, tag="u")
    nc.gpsimd.tensor_scalar(u[:], t2[:], -1.0, 1.0, ALU.mult, ALU.add)
    w = tb.tile([64, 512], F32, tag="w")
    nc.vector.tensor_mul(w[:], c2t[:], u[:])
    d1 = tb.tile([64, 512], F32, tag="d1")
    nc.vector.tensor_sub(d1[:], w[:], u[:])
    dd = tb.tile([64, 512], F32, tag="dd")
    nc.gpsimd.tensor_scalar_add(dd[:], d1[:], 2.0)
    r = tb.tile([64, 512], F32, tag="r")
    nc.vector.reciprocal(r[:], dd[:])
    tr = tb.tile([64, 512], F32, tag="tr")
    nc.vector.tensor_mul(tr[:], t[:], r[:])
    su = tb.tile([64, 512], F32, tag="su")
    nc.vector.tensor_mul(su[:], s2[:], u[:])
    nsu = tb.tile([64, 512], F32, tag="nsu")
    nc.gpsimd.tensor_scalar_mul(nsu[:], su[:], -1.0)
    # pack writes (fp16)
    nc.scalar.activation(pack_re[0:64, :], tr[:], AF.Copy, bias=0.0, scale=2.0)
    nc.scalar.activation(pack_im[64:128, :], tr[:], AF.Copy, bias=0.0,
                         scale=2.0)
    nc.vector.tensor_mul(pack_im[0:64, :], su[:], r[:])
    nc.vector.tensor_mul(pack_re[64:128, :], nsu[:], r[:])


def _build():
    nc = bacc.Bacc("TRN2", target_bir_lowering=False, debug=False)

    def reg_const(value, dtype=F32):
        t = nc.alloc_sbuf_tensor(f"const-{dtype.name}-{value}", [128, 1], dtype)
        nc.gpsimd.memset(t.ap(), value)
        nc.const_aps.aps[(dtype, value)] = t.ap()

    reg_const(HALF_PI)
    nc.all_engine_barrier()

    c2hi_np, c2lo_np, cinv_np = _dft_consts()
    C2H = nc.inline_tensor(np.ascontiguousarray(c2hi_np), name="C2H")
    C2L = nc.inline_tensor(np.ascontiguousarray(c2lo_np), name="C2L")
    CINV = nc.inline_tensor(np.ascontiguousarray(cinv_np), name="CINV")

    QK = nc.dram_tensor("qk", [B, 128, 2, NCH, 128], F16, kind="ExternalInput")
    WP = nc.dram_tensor("wp", [128, M, 128], F16, kind="ExternalInput")
    OUT = nc.dram_tensor("out", [B, E, L], F16, kind="ExternalOutput")

    with tile.TileContext(nc) as tc, ExitStack() as ctx:
        cpool = ctx.enter_context(tc.tile_pool(name="consts", bufs=1))
        qk_pool = ctx.enter_context(tc.tile_pool(name="qk", bufs=6))
        ft_pool = ctx.enter_context(tc.tile_pool(name="ft", bufs=4))
        ktr_pool = ctx.enter_context(tc.tile_pool(name="ktr", bufs=16))
        tb_pool = ctx.enter_context(tc.tile_pool(name="tanh", bufs=2))
        st_pool = ctx.enter_context(tc.tile_pool(name="stage", bufs=3))
        ps_grp = ctx.enter_context(
            tc.tile_pool(name="ps_grp", bufs=6, space="PSUM"))
        ps_sm = ctx.enter_context(
            tc.tile_pool(name="ps_sm", bufs=2, space="PSUM"))

        c2h = cpool.tile([128, NCH, 128], F16)
        nc.gpsimd.dma_start(c2h[:], C2H[:])
        c2l = cpool.tile([128, NCH, 128], F16)
        cinv = cpool.tile([128, L], F16)
        wp = cpool.tile([128, M, 128], F16)
        ident = cpool.tile([128, 128], F16)
        masks.make_identity(nc, ident[:])

        ktrs = [None] * B

        e1_tiles = []
        for g in range(2):
            ps_e1_re = ps_grp.tile([64, 512], F32, tag="grp")
            ps_e1_im = ps_grp.tile([64, 512], F32, tag="grp")
            e1_tiles.append((ps_e1_re, ps_e1_im))
            for j in range(8):
                b = g * 8 + j
                # ---------- forward DFT ----------
                qkt = qk_pool.tile([128, 2, NCH, 128], F16, tag="qkt")
                nc.gpsimd.dma_start(qkt[:], QK[b])
                if b == 0:
                    nc.gpsimd.dma_start(c2l[:], C2L[:])
                elif b == 2:
                    nc.gpsimd.dma_start(wp[:], WP[:])
                    nc.gpsimd.dma_start(cinv[:], CINV[:])
                qkh = qkt[:, 0]
                qkl = qkt[:, 1]

                ps_f = ps_grp.tile([128, 128], F32, tag="grp")
                for n in range(NCH):
                    nc.tensor.matmul(ps_f[:], qkh[:, n], c2h[:, n, :],
                                     start=(n == 0), stop=False)
                    nc.tensor.matmul(ps_f[:], qkh[:, n], c2l[:, n, :],
                                     start=False, stop=False)
                    nc.tensor.matmul(ps_f[:], qkl[:, n], c2h[:, n, :],
                                     start=False, stop=(n == NCH - 1))

                # ---------- hi/lo splits ----------
                ftqH = ft_pool.tile([64, 128], F16, tag="ftqH")
                nc.scalar.copy(ftqH[:], ps_f[0:64, :])
                ftqL = ft_pool.tile([64, 128], F16, tag="ftqL")
                nc.vector.tensor_sub(ftqL[:], ps_f[0:64, :], ftqH[:])
                ftqN = ft_pool.tile([64, 128], F16, tag="ftqN")
                nc.gpsimd.tensor_scalar_mul(ftqN[:, 0:64], ftqH[:, 64:128],
                                            -1.0)
                nc.gpsimd.tensor_scalar_mul(ftqN[:, 64:128], ftqL[:, 64:128],
                                            -1.0)

                kst = ft_pool.tile([64, 128], F32, tag="kst")
                nc.scalar.copy(kst[:], ps_f[64:128, :])
                ftkH = ft_pool.tile([64, 128], F16, tag="ftkH")
                nc.gpsimd.tensor_copy(ftkH[:], kst[:])
                ftkL = ft_pool.tile([64, 128], F16, tag="ftkL")
                nc.vector.tensor_sub(ftkL[:], kst[:], ftkH[:])

                # ---------- ktr = [KreT ; KimT] [128, 64] for e2 ----------
                ps_t = ps_sm.tile([128, 64], F16, tag="sm")
                nc.tensor.transpose(ps_t[:], ftkH[:], ident[0:64, 0:64])
                ktr = ktr_pool.tile([128, 64], F16, tag="ktr")
                nc.vector.tensor_copy(ktr[:], ps_t[:])
                ktrs[b] = ktr

                # ---------- e1 ----------
                col = j * 64
                out_re = ps_e1_re[:, col:col + 64]
                out_im = ps_e1_im[:, col:col + 64]
                KreH, KimH = ftkH[:, 0:64], ftkH[:, 64:128]
                KreL, KimL = ftkL[:, 0:64], ftkL[:, 64:128]
                QreH, QimH = ftqH[:, 0:64], ftqH[:, 64:128]
                QreL, QimL = ftqL[:, 0:64], ftqL[:, 64:128]
                nQimH, nQimL = ftqN[:, 0:64], ftqN[:, 64:128]
                nc.tensor.matmul(out_re, KreH, QreH, start=True, stop=False)
                nc.tensor.matmul(out_re, KreH, QreL, start=False, stop=False)
                nc.tensor.matmul(out_im, KreH, QimH, start=True, stop=False)
                nc.tensor.matmul(out_im, KreH, QimL, start=False, stop=False)
                nc.tensor.matmul(out_re, KimH, nQimH, start=False, stop=False)
                nc.tensor.matmul(out_re, KimH, nQimL, start=False, stop=False)
                nc.tensor.matmul(out_im, KimH, QreH, start=False, stop=False)
                nc.tensor.matmul(out_im, KimH, QreL, start=False, stop=False)
                nc.tensor.matmul(out_re, KreL, QreH, start=False, stop=False)
                nc.tensor.matmul(out_im, KreL, QimH, start=False, stop=False)
                nc.tensor.matmul(out_re, KimL, nQimH, start=False, stop=True)
                nc.tensor.matmul(out_im, KimL, QreH, start=False, stop=True)

        for g in range(2):
            ps_e1_re, ps_e1_im = e1_tiles[g]
            # ---------- tanh + packs for this group ----------
            pack_re = tb_pool.tile([128, 512], F16, tag="pack_re")
            pack_im = tb_pool.tile([128, 512], F16, tag="pack_im")
            _tanh_block(nc, tb_pool, ps_e1_re, ps_e1_im, pack_re, pack_im, g)

            # ---------- e2 ----------
            ps_v_re = ps_grp.tile([64, 512], F32, tag="grp")
            ps_v_im = ps_grp.tile([64, 512], F32, tag="grp")
            for j in range(8):
                b = g * 8 + j
                col = j * 64
                nc.tensor.matmul(ps_v_re[:, col:col + 64], ktrs[b][:],
                                 pack_re[:, col:col + 64],
                                 start=True, stop=True)
                nc.tensor.matmul(ps_v_im[:, col:col + 64], ktrs[b][:],
                                 pack_im[:, col:col + 64],
                                 start=True, stop=True)

            # Vpack_g [e_re|e_im, (b%8)*64+x]
            vpack = tb_pool.tile([128, 512], F16, tag="vpack")
            nc.scalar.copy(vpack[0:64, :], ps_v_re[:])
            nc.vector.tensor_copy(vpack[64:128, :], ps_v_im[:])

            # ---------- wmix (this group: N=8 batches) ----------
            ps_w = ps_grp.tile([128, 512], F32, tag="grp")
            vp3 = vpack[:].rearrange("p (b x) -> p b x", x=64)
            for x in range(M):
                nc.tensor.matmul(ps_w[:, x * 8:(x + 1) * 8], wp[:, x, :],
                                 vp3[:, :, x], start=True, stop=True)

            xs = tb_pool.tile([128, 512], F16, tag="xs")
            nc.scalar.activation(xs[:], ps_w[:], AF.Copy, bias=0.0, scale=S_X)
            xs3 = xs[:].rearrange("p (x b) -> p x b", b=8)

            # ---------- inverse DFT (this group) ----------
            for bp in range(4):
                lhs = st_pool.tile([128, 128], F16, tag="lhs_inv")
                for j in range(2):
                    bb = 2 * bp + j
                    ps_xt = ps_sm.tile([64, 128], F16, tag="sm")
                    nc.tensor.transpose(ps_xt[:], xs3[:, :, bb], ident[:])
                    nc.scalar.copy(lhs[0:64, j * 64:(j + 1) * 64],
                                   ps_xt[:, 0:64])
                    nc.vector.tensor_copy(lhs[64:128, j * 64:(j + 1) * 64],
                                          ps_xt[:, 64:128])
                ot = st_pool.tile([128, 2048], F16, tag="ostage")
                for c in range(4):
                    ps_o = ps_grp.tile([128, 512], F32, tag="grp")
                    nc.tensor.matmul(ps_o[:], lhs[:],
                                     cinv[:, c * 512:(c + 1) * 512],
                                     start=True, stop=True)
                    if c % 2 == 0:
                        nc.scalar.copy(ot[:, c * 512:(c + 1) * 512], ps_o[:])
                    else:
                        nc.vector.tensor_copy(ot[:, c * 512:(c + 1) * 512],
                                              ps_o[:])
                bo = g * 8 + 2 * bp
                nc.sync.dma_start(OUT[bo:bo + 2, :, :], ot[:])

    nc.compile()
    return nc


def _host_pack(q, k, w_re, w_im):
    in_maps = []
    for h in range(H):
        qT = np.ascontiguousarray(q[:, :, h, :].transpose(0, 2, 1))
        kT = np.ascontiguousarray(k[:, :, h, :].transpose(0, 2, 1))
        qk = np.concatenate([qT, kT], axis=2)          # [B, L, 128]
        qkh = qk.astype(np.float16)
        qkl = (qk - qkh.astype(np.float32)).astype(np.float16)
        qkhl = np.stack([qkh.reshape(B, 128, NCH, 128),
                         qkl.reshape(B, 128, NCH, 128)], axis=2)

        wre = (w_re[h] * S_W).astype(np.float32)       # [E, E(o), M]
        wim = (w_im[h] * S_W).astype(np.float32)
        wp = np.empty((128, M, 128), dtype=np.float16)
        wp[0:64, :, 0:64] = wre.transpose(0, 2, 1)
        wp[64:128, :, 0:64] = -wim.transpose(0, 2, 1)
        wp[0:64, :, 64:128] = wim.transpose(0, 2, 1)
        wp[64:128, :, 64:128] = wre.transpose(0, 2, 1)
        in_maps.append({
            "qk": np.ascontiguousarray(qkhl),
            "wp": wp,
        })
    return in_maps


def kernel(q, k, v, w_re, w_im, _trace=False):
    q = np.asarray(q, dtype=np.float32)
    k = np.asarray(k, dtype=np.float32)
    w_re = np.asarray(w_re, dtype=np.float32)
    w_im = np.asarray(w_im, dtype=np.float32)

    if "nc" not in _CACHE:
        _CACHE["nc"] = _build()
    nc = _CACHE["nc"]

    in_maps = _host_pack(q, k, w_re, w_im)
    res = None
    for attempt in range(3):
        try:
            res = run_bass_kernel_spmd(nc, in_maps, list(range(H)),
                                       trace=_trace)
            break
        except Exception:
            if attempt == 2:
                raise
            import time as _time
            _time.sleep(5.0)
    out = np.stack([res.results[h]["out"].astype(np.float32) * S_HOST
                    for h in range(H)], axis=2)
    _CACHE["last_result"] = res
    return np.ascontiguousarray(out)  # [B, E, H, L]



# revision 37
# speedup vs baseline: 1.1854x; 1.1854x over previous
"""FourierCrossAttention Trainium2 kernel.

Sharding: one head per NeuronCore (H=8, n_cores=8); each core processes all
B=16 batches for its head.

Math (per (b,h)):
  ftq = qT @ [cos | -sin]           (DFT, first 64 modes; fp16 hi/lo split)
  ftk = kT @ [cos | -sin]
  xy[y,x]   = sum_e ftk[e,y]*ftq[e,x]      (complex)
  A = tanh(xy)                             (complex tanh, stable form)
  v[e,x]    = sum_y ftk[e,y]*A[x,y]        (complex)
  X[o,x]    = sum_e v[e,x]*W[e,o,x]        (complex, per-head weights)
  out[o,l]  = sum_x Re(X)*cr[x,l] + Im(X)*ci[x,l]   (inverse rDFT, /(E*E))

Pipeline: 4 groups of 4 batches, software-pipelined 2 groups deep so each
in-order engine queue always has ready work: tanh(g) runs under DFT(g+1),
inv/out(g) under DFT(g+2). Act-table loads are prefetched by dummy ops.
"""
import sys
sys.path.insert(0, '/opt/trn_rl_repo')
import numpy as np
from contextlib import ExitStack

import concourse.bacc as bacc
import concourse.mybir as mybir
import concourse.tile as tile
from concourse import masks
from concourse.bass_utils import run_bass_kernel_spmd

F32 = mybir.dt.float32
F16 = mybir.dt.float16
AF = mybir.ActivationFunctionType
ALU = mybir.AluOpType

B, E, H, L = 16, 64, 8, 2048
M = 64                      # modes
NCH = 16                    # l-chunks of 128
GRP = 4                     # batches per pipeline group
S_W = 4096.0                # weight prescale (2^12)
S_X = 2.0 ** -18            # X downcast scale (keeps Xs out of fp16 subnormals)
S_OUT = 2.0 ** 14           # 2^-6 remainder * 2^20 fp16-range boost
S_HOST = 2.0 ** -21         # host-side unscale (incl. 1/2 for doubled k)
S_C = 2.0 ** -11            # folded into Cinv (1/L)
# S_W * S_X * S_C * S_OUT = 2^12 * 2^-18 * 2^-11 * 2^-6 = 2^-23 = 1/(L*E*E)

INV2PI = float(1.0 / (2 * np.pi))
MAGIC = 12582912.0          # 1.5 * 2^23 round-to-int magic
C1 = 6.28125                # Cody-Waite 2pi hi (exact in 9 bits)
C2W = float(2 * np.pi - 6.28125)

_CACHE = {}
_DEBUG = False
_MARKS = []  # (label, "I-<n>") build-order markers for trace attribution

# Route Tanh AND Sin to the one act-func set that holds both
# (silu_and_others), so the whole kernel needs a single 1.3us table load
# instead of two per tanh block.
_orig_gat = bacc.get_activation_tables


def _gat_silu_only(arch):
    tables = _orig_gat(arch)
    for name, funcs in tables.items():
        if name != 'silu_and_others':
            funcs.discard(AF.Tanh)
            funcs.discard(AF.Sin)
    return tables


bacc.get_activation_tables = _gat_silu_only


def _dft_consts():
    l = np.arange(L)[:, None]
    m = np.arange(M)[None, :]
    ang = 2.0 * np.pi * l * m / L
    c2 = np.concatenate([np.cos(ang), -np.sin(ang)], axis=1)   # [L, 128]
    c2hi = c2.astype(np.float16)
    c2lo = (c2 - c2hi.astype(np.float64)).astype(np.float16)
    c2hi = c2hi.reshape(128, NCH, 128)
    c2lo = c2lo.reshape(128, NCH, 128)

    x = np.arange(M)[:, None]
    lr = np.arange(L)[None, :]
    ang2 = 2.0 * np.pi * x * lr / L
    dup = np.where(np.arange(M) == 0, 1.0, 2.0)[:, None]
    cr = dup * np.cos(ang2) * (S_C * S_OUT)
    ci = -dup * np.sin(ang2) * (S_C * S_OUT)
    ci[0, :] = 0.0
    cinv = np.concatenate([cr, ci], axis=0).astype(np.float16)  # [128, L]
    return c2hi, c2lo, cinv


def _tanh_block(nc, tb, e1, pack_re, pack_im):
    """Complex tanh of e1 = [64, re 0:256 | im 256:512] psum; writes the
    e2 rhs packs: pack_re=[2t/D ; -Im], pack_im=[Im ; 2t/D] as fp16.

    The serial chain lives on DVE (in-order, no sem hops); Act only does
    the three transcendentals; Pool takes off-critical-path scalar ops.
    Dummy act ops prefetch the tanh/trig tables off the critical path.
    """
    W = 256
    ps_re = e1[:, 0:W]
    ps_im = e1[:, W:2 * W]
    t = tb.tile([64, W], F32, tag="t")
    # k (and hence e1) is doubled host-side: tanh(re) = Tanh(ps_re * 0.5)
    nc.scalar.activation(t[:], ps_re, AF.Tanh, scale=0.5)
    # range reduction of z = ps_im mod 2pi (Cody-Waite); ps_im is already 2*Im
    u1 = tb.tile([64, W], F32, tag="u1")
    nc.vector.tensor_scalar(u1[:], ps_im, INV2PI, MAGIC,
                            ALU.mult, ALU.add)
    w1 = tb.tile([64, W], F32, tag="w1")
    nc.vector.tensor_scalar(w1[:], u1[:], MAGIC, C1, ALU.subtract, ALU.mult)
    w2 = tb.tile([64, W], F32, tag="w2")
    nc.gpsimd.tensor_scalar(w2[:], u1[:], MAGIC, C2W, ALU.subtract, ALU.mult)
    zr0 = tb.tile([64, W], F32, tag="zr0")
    nc.vector.tensor_sub(zr0[:], ps_im, w1[:])
    zr = tb.tile([64, W], F32, tag="zr")
    nc.vector.tensor_sub(zr[:], zr0[:], w2[:])
    s2 = tb.tile([64, W], F32, tag="s2")
    nc.scalar.activation(s2[:], zr[:], AF.Sin)
    sh = tb.tile([64, W], F32, tag="sh")
    nc.scalar.activation(sh[:], zr[:], AF.Sin, scale=0.5)
    t2 = tb.tile([64, W], F32, tag="t2")
    nc.vector.tensor_mul(t2[:], t[:], t[:])
    u = tb.tile([64, W], F32, tag="u")
    nc.gpsimd.tensor_scalar(u[:], t2[:], -1.0, 1.0, ALU.mult, ALU.add)
    sh2 = tb.tile([64, W], F32, tag="sh2")
    nc.vector.tensor_mul(sh2[:], sh[:], sh[:])
    # D/2 = 1 - u*sin^2(zr/2)  (exactly (2 - u + cos(zr)*u)/2); r = 2/D
    su = tb.tile([64, W], F32, tag="su")
    nc.vector.tensor_mul(su[:], s2[:], u[:])
    srh = tb.tile([64, W], F32, tag="srh")
    nc.gpsimd.tensor_scalar_mul(srh[:], su[:], 0.5)
    nsrh = tb.tile([64, W], F32, tag="nsrh")
    nc.gpsimd.tensor_scalar_mul(nsrh[:], su[:], -0.5)
    nsh = tb.tile([64, W], F32, tag="nsh")
    nc.vector.tensor_mul(nsh[:], u[:], sh2[:])
    dd2 = tb.tile([64, W], F32, tag="dd2")
    nc.vector.tensor_scalar(dd2[:], nsh[:], -1.0, 1.0, ALU.mult, ALU.add)
    r = tb.tile([64, W], F32, tag="r")
    nc.vector.reciprocal(r[:], dd2[:])
    # pack writes (fp16): pack_re=[2t/D ; -Im], pack_im=[Im ; 2t/D]
    nc.vector.tensor_mul(pack_re[0:64, :], t[:], r[:])
    nc.gpsimd.tensor_copy(pack_im[64:128, :], pack_re[0:64, :])
    nc.vector.tensor_mul(pack_im[0:64, :], srh[:], r[:])
    nc.gpsimd.tensor_mul(pack_re[64:128, :], nsrh[:], r[:])


def _build():
    nc = bacc.Bacc("TRN2", target_bir_lowering=False, debug=False)

    c2hi_np, c2lo_np, cinv_np = _dft_consts()
    C2H = nc.inline_tensor(np.ascontiguousarray(c2hi_np), name="C2H")
    C2L = nc.inline_tensor(np.ascontiguousarray(c2lo_np), name="C2L")
    CINV = nc.inline_tensor(np.ascontiguousarray(cinv_np), name="CINV")

    QK = nc.dram_tensor("qk", [B, 128, 2, NCH, 128], F16, kind="ExternalInput")
    WP = nc.dram_tensor("wp", [128, M, 128], F16, kind="ExternalInput")
    OUT = nc.dram_tensor("out", [B, E, L], F16, kind="ExternalOutput")
    DBG_E1 = nc.dram_tensor("dbg_e1", [4, 64, 512], F32, kind="ExternalOutput")
    DBG_PR = nc.dram_tensor("dbg_pr", [4, 128, 256], F16,
                            kind="ExternalOutput")
    DBG_PI = nc.dram_tensor("dbg_pi", [4, 128, 256], F16,
                            kind="ExternalOutput")
    DBG_VP = nc.dram_tensor("dbg_vp", [4, 128, 256], F16,
                            kind="ExternalOutput")
    DBG_XS = nc.dram_tensor("dbg_xs", [4, 128, 256], F16,
                            kind="ExternalOutput")
    DBG_FT = nc.dram_tensor("dbg_ft", [B, 2, 64, 128], F16,
                            kind="ExternalOutput")

    with tile.TileContext(nc) as tc, ExitStack() as ctx:
        cpool = ctx.enter_context(tc.tile_pool(name="consts", bufs=1))
        qk_pool = ctx.enter_context(tc.tile_pool(name="qk", bufs=10))
        ft_pool = ctx.enter_context(tc.tile_pool(name="ft", bufs=3))
        ktr_pool = ctx.enter_context(tc.tile_pool(name="ktr", bufs=16))
        tb_pool = ctx.enter_context(tc.tile_pool(name="tanh", bufs=2))
        st_pool = ctx.enter_context(tc.tile_pool(name="stage", bufs=3))
        # PSUM budget (8 banks): pf 2 + pe1 2 + pv 1 + pwo 2 + psm 1
        pf_pool = ctx.enter_context(
            tc.tile_pool(name="pf", bufs=2, space="PSUM"))
        pe1_pool = ctx.enter_context(
            tc.tile_pool(name="pe1", bufs=2, space="PSUM"))
        pv_pool = ctx.enter_context(
            tc.tile_pool(name="pv", bufs=1, space="PSUM"))
        pwo_pool = ctx.enter_context(
            tc.tile_pool(name="pwo", bufs=2, space="PSUM"))
        psm_pool = ctx.enter_context(
            tc.tile_pool(name="psm", bufs=1, space="PSUM"))

        c2h = cpool.tile([128, NCH, 128], F16)
        nc.sync.dma_start(c2h[:], C2H[:])
        c2l = cpool.tile([128, NCH, 128], F16)
        cinv = cpool.tile([128, L], F16)
        wp = cpool.tile([128, M, 128], F16)
        ident = cpool.tile([128, 128], F16)
        masks.make_identity(nc, ident[:])

        ktrs = [None] * B
        e1s = {}
        packs = {}
        vpacks = {}
        xss = {}
        lhss = {}

        def mark(lbl):
            _MARKS.append((lbl, nc.get_next_instruction_name()))

        qkts = [None] * B

        def emit_qk_dma(b):
            qkt = qk_pool.tile([128, 2, NCH, 128], F16, tag="qkt",
                               name="qkt")
            nc.sync.dma_start(qkt[:], QK[b])
            if b == 0:
                nc.sync.dma_start(c2l[:], C2L[:])
            elif b == 1:
                nc.sync.dma_start(wp[:], WP[:])
            elif b == 2:
                nc.sync.dma_start(cinv[:], CINV[:])
            qkts[b] = qkt

        def emit_dft_batch(g, j):
            b = g * GRP + j
            mark(f"dft({g},{j})")
            # ---------- forward DFT ----------
            qkt = qkts[b]
            qkh = qkt[:, 0]
            qkl = qkt[:, 1]

            ps_f = pf_pool.tile([128, 128], F32, tag="f")
            for n in range(NCH):
                nc.tensor.matmul(ps_f[:], qkh[:, n], c2h[:, n, :],
                                 start=(n == 0), stop=False)
                nc.tensor.matmul(ps_f[:], qkh[:, n], c2l[:, n, :],
                                 start=False, stop=False)
                nc.tensor.matmul(ps_f[:], qkl[:, n], c2h[:, n, :],
                                 start=False, stop=(n == NCH - 1))

            # ---------- hi/lo splits ----------
            # k-half staged to base-0 SBUF (TensorTensor inputs must share
            # a start partition); q-half splits read the psum directly
            ftqH = ft_pool.tile([64, 128], F16, tag="ftqH")
            nc.scalar.copy(ftqH[:], ps_f[0:64, :])
            ftqL = ft_pool.tile([64, 128], F16, tag="ftqL")
            nc.vector.tensor_sub(ftqL[:], ps_f[0:64, :], ftqH[:])
            ftqN = ft_pool.tile([64, 128], F16, tag="ftqN")
            nc.gpsimd.tensor_scalar_mul(ftqN[:, 0:64], ftqH[:, 64:128], -1.0)
            nc.gpsimd.tensor_scalar_mul(ftqN[:, 64:128], ftqL[:, 64:128],
                                        -1.0)
            kst = ft_pool.tile([64, 128], F32, tag="kst")
            nc.scalar.copy(kst[:], ps_f[64:128, :])
            ftkH = ft_pool.tile([64, 128], F16, tag="ftkH")
            nc.gpsimd.tensor_copy(ftkH[:], kst[:])
            ftkL = ft_pool.tile([64, 128], F16, tag="ftkL")
            nc.vector.tensor_sub(ftkL[:], kst[:], ftkH[:])
            if _DEBUG:
                nc.sync.dma_start(DBG_FT[b, 0], ftqH[:])
                nc.sync.dma_start(DBG_FT[b, 1], ftkH[:])

            # ---------- ktr = [KreT ; KimT] [128, 64] for e2 ----------
            ps_t = psm_pool.tile([128, 64], F16, tag="sm", name="ps_t")
            nc.tensor.transpose(ps_t[:], ftkH[:], ident[0:64, 0:64])
            ktr = ktr_pool.tile([128, 64], F16, tag="ktr")
            nc.vector.tensor_copy(ktr[:], ps_t[:])
            ktrs[b] = ktr

            # ---------- e1 ----------
            e1 = e1s[g]
            col = j * 64
            out_re = e1[:, col:col + 64]
            out_im = e1[:, 256 + col:256 + col + 64]
            KreH, KimH = ftkH[:, 0:64], ftkH[:, 64:128]
            KreL, KimL = ftkL[:, 0:64], ftkL[:, 64:128]
            QreH, QimH = ftqH[:, 0:64], ftqH[:, 64:128]
            QreL, QimL = ftqL[:, 0:64], ftqL[:, 64:128]
            nQimH, nQimL = ftqN[:, 0:64], ftqN[:, 64:128]
            # one pending accumulation stream per psum bank at a time: a
            # start=True wipes the bank's un-stopped partials on hardware,
            # so re must fully commit (stop) before im starts
            nc.tensor.matmul(out_re, KreH, QreH, start=True, stop=False)
            nc.tensor.matmul(out_re, KreH, QreL, start=False, stop=False)
            nc.tensor.matmul(out_re, KimH, nQimH, start=False, stop=False)
            nc.tensor.matmul(out_re, KimH, nQimL, start=False, stop=False)
            nc.tensor.matmul(out_re, KreL, QreH, start=False, stop=False)
            nc.tensor.matmul(out_re, KimL, nQimH, start=False, stop=True)
            nc.tensor.matmul(out_im, KreH, QimH, start=True, stop=False)
            nc.tensor.matmul(out_im, KreH, QimL, start=False, stop=False)
            nc.tensor.matmul(out_im, KimH, QreH, start=False, stop=False)
            nc.tensor.matmul(out_im, KimH, QreL, start=False, stop=False)
            nc.tensor.matmul(out_im, KreL, QimH, start=False, stop=False)
            nc.tensor.matmul(out_im, KimL, QreH, start=False, stop=True)

        def emit_tanh(g):
            mark(f"tanh({g})")
            if _DEBUG:
                e1st = tb_pool.tile([64, 512], F32, tag="e1st", name="e1st")
                nc.scalar.copy(e1st[:], e1s[g][:])
                nc.sync.dma_start(DBG_E1[g], e1st[:])
            pack_re = tb_pool.tile([128, 256], F16, tag="pack_re")
            pack_im = tb_pool.tile([128, 256], F16, tag="pack_im")
            _tanh_block(nc, tb_pool, e1s[g], pack_re, pack_im)
            packs[g] = (pack_re, pack_im)
            if _DEBUG:
                nc.sync.dma_start(DBG_PR[g], pack_re[:])
                nc.sync.dma_start(DBG_PI[g], pack_im[:])

        def emit_e2(g):
            mark(f"e2({g})")
            pack_re, pack_im = packs[g]
            pv = pv_pool.tile([64, 512], F32, tag="v")
            for j in range(GRP):
                b = g * GRP + j
                col = j * 64
                nc.tensor.matmul(pv[:, col:col + 64], ktrs[b][:],
                                 pack_re[:, col:col + 64],
                                 start=True, stop=True)
                nc.tensor.matmul(pv[:, 256 + col:256 + col + 64], ktrs[b][:],
                                 pack_im[:, col:col + 64],
                                 start=True, stop=True)

            # vpack [e_re|e_im, b*64+x]
            vpack = st_pool.tile([128, 256], F16, tag="vpack")
            nc.scalar.copy(vpack[0:64, :], pv[:, 0:256])
            nc.vector.tensor_copy(vpack[64:128, :], pv[:, 256:512])
            vpacks[g] = vpack
            if _DEBUG:
                nc.sync.dma_start(DBG_VP[g], vpack[:])

        def emit_wmix(g):
            mark(f"wmix({g})")
            vpack = vpacks[g]
            ps_w = pwo_pool.tile([128, 256], F32, tag="wo")
            vp3 = vpack[:].rearrange("p (b x) -> p b x", x=64)
            for x in range(M):
                nc.tensor.matmul(ps_w[:, x * GRP:(x + 1) * GRP], wp[:, x, :],
                                 vp3[:, :, x], start=True, stop=True)

            xs = st_pool.tile([128, 256], F16, tag="xs")
            nc.scalar.activation(xs[:], ps_w[:], AF.Copy, bias=0.0, scale=S_X)
            xss[g] = xs
            if _DEBUG:
                nc.sync.dma_start(DBG_XS[g], xs[:])

        def emit_xt(g):
            mark(f"xt({g})")
            xs3 = xss[g][:].rearrange("p (x b) -> p x b", b=GRP)
            pair_lhs = []
            for bp in range(GRP // 2):
                lhs = st_pool.tile([128, 128], F16, tag="lhs_inv")
                for j in range(2):
                    bb = 2 * bp + j
                    ps_xt = pv_pool.tile([128, 64], F16, tag="v",
                                         name="ps_xt")
                    nc.tensor.transpose(ps_xt[0:64, :], xs3[0:64, :, bb],
                                        ident[0:64, 0:64])
                    nc.tensor.transpose(ps_xt[64:128, :], xs3[64:128, :, bb],
                                        ident[64:128, 64:128])
                    if j == 0:
                        nc.scalar.copy(lhs[:, 0:64], ps_xt[:])
                    else:
                        nc.vector.tensor_copy(lhs[:, 64:128], ps_xt[:])
                pair_lhs.append(lhs)
            lhss[g] = pair_lhs

        def emit_inv_pair(g, bp):
            mark(f"inv({g},{bp})")
            lhs = lhss[g][bp]
            bo = g * GRP + 2 * bp
            ov = OUT[bo:bo + 2, :, :].rearrange("b o l -> (b o) l")
            ot = st_pool.tile([128, 2048], F16, tag="ostage")
            for c in range(4):
                ps_o = pwo_pool.tile([128, 512], F32, tag="wo")
                nc.tensor.matmul(ps_o[:], lhs[:],
                                 cinv[:, c * 512:(c + 1) * 512],
                                 start=True, stop=True)
                if c % 2 == 0:
                    nc.scalar.copy(ot[:, c * 512:(c + 1) * 512], ps_o[:])
                else:
                    nc.vector.tensor_copy(ot[:, c * 512:(c + 1) * 512],
                                          ps_o[:])
                # chunked store rides the otherwise-idle SP sequencer; all
                # qk loads were emitted in the prologue so no HOL risk
                nc.sync.dma_start(ov[:, c * 512:(c + 1) * 512],
                                  ot[:, c * 512:(c + 1) * 512])

        # PE warmup: dummy matmuls while the first QK load is in flight, so
        # the tensor engine is past its p-state ramp when real work arrives.
        wu = pwo_pool.tile([128, 512], F32, tag="wo", name="wu")
        for i in range(20):
            nc.tensor.matmul(wu[:, 0:128], ident[:], ident[:],
                             start=(i == 0), stop=(i == 19))
        # Act-table warmup: the single tanh+sin set loads once, off-path
        wud = tb_pool.tile([64, 1], F32, tag="dummy", name="wud")
        nc.scalar.activation(wud[:], ident[0:64, 0:1], AF.Tanh)

        # Software pipeline, 2 groups deep: group g's stages are spread
        # under DFT(g+1) (tanh/e2/wmix/xt) and DFT(g+2) (inv+out) so the
        # in-order engine queues never head-of-line-block the next group.
        for b in range(B):
            emit_qk_dma(b)

        NG = B // GRP
        stages = {}

        def at(slot, fn):
            stages.setdefault(slot, []).append(fn)

        for g in range(NG):
            at((g + 1, 0), lambda g=g: emit_tanh(g))
            at((g + 1, 3), lambda g=g: emit_e2(g))
        for g in range(NG):
            at((g + 2, 0), lambda g=g: emit_wmix(g))
            at((g + 2, 1), lambda g=g: emit_xt(g))
            at((g + 2, 2), lambda g=g: emit_inv_pair(g, 0))
            at((g + 2, 3), lambda g=g: emit_inv_pair(g, 1))

        for g in range(NG + 2):
            if g < NG:
                e1s[g] = pe1_pool.tile([64, 512], F32, tag="e1", name="e1")
            for fn in stages.get((g, 0), []):
                fn()
            for j in range(GRP):
                if g < NG:
                    emit_dft_batch(g, j)
                if j > 0:
                    for fn in stages.get((g, j), []):
                        fn()

    nc.compile()
    return nc


def _host_pack(q, k, w_re, w_im):
    in_maps = []
    for h in range(H):
        qT = np.ascontiguousarray(q[:, :, h, :].transpose(0, 2, 1))
        kT = np.ascontiguousarray(k[:, :, h, :].transpose(0, 2, 1))
        qk = np.concatenate([qT, 2.0 * kT], axis=2)    # [B, L, 128]
        qkh = qk.astype(np.float16)
        qkl = (qk - qkh.astype(np.float32)).astype(np.float16)
        qkhl = np.stack([qkh.reshape(B, 128, NCH, 128),
                         qkl.reshape(B, 128, NCH, 128)], axis=2)

        wre = (w_re[h] * S_W).astype(np.float32)       # [E, E(o), M]
        wim = (w_im[h] * S_W).astype(np.float32)
        wp = np.empty((128, M, 128), dtype=np.float16)
        wp[0:64, :, 0:64] = wre.transpose(0, 2, 1)
        wp[64:128, :, 0:64] = -wim.transpose(0, 2, 1)
        wp[0:64, :, 64:128] = wim.transpose(0, 2, 1)
        wp[64:128, :, 64:128] = wre.transpose(0, 2, 1)
        in_maps.append({
            "qk": np.ascontiguousarray(qkhl),
            "wp": wp,
        })
    return in_maps


def kernel(q, k, v, w_re, w_im, _trace=False):
    q = np.asarray(q, dtype=np.float32)
    k = np.asarray(k, dtype=np.float32)
    w_re = np.asarray(w_re, dtype=np.float32)
    w_im = np.asarray(w_im, dtype=np.float32)

    if "nc" not in _CACHE:
        _CACHE["nc"] = _build()
    nc = _CACHE["nc"]

    in_maps = _host_pack(q, k, w_re, w_im)
    res = None
    for attempt in range(3):
        try:
            res = run_bass_kernel_spmd(nc, in_maps, list(range(H)),
                                       trace=_trace)
            break
        except Exception:
            if attempt == 2:
                raise
            import time as _time
            _time.sleep(5.0)
    out = np.stack([res.results[h]["out"].astype(np.float32) * S_HOST
                    for h in range(H)], axis=2)
    _CACHE["last_result"] = res
    return np.ascontiguousarray(out)  # [B, E, H, L]


# revision 38
# speedup vs baseline: 1.2296x; 1.0374x over previous
"""FourierCrossAttention Trainium2 kernel.

Sharding: one head per NeuronCore (H=8, n_cores=8); each core processes all
B=16 batches for its head.

Math (per (b,h)):
  ftq = qT @ [cos | -sin]           (DFT, first 64 modes; fp16 hi/lo split)
  ftk = kT @ [cos | -sin]
  xy[y,x]   = sum_e ftk[e,y]*ftq[e,x]      (complex)
  A = tanh(xy)                             (complex tanh, stable form)
  v[e,x]    = sum_y ftk[e,y]*A[x,y]        (complex)
  X[o,x]    = sum_e v[e,x]*W[e,o,x]        (complex, per-head weights)
  out[o,l]  = sum_x Re(X)*cr[x,l] + Im(X)*ci[x,l]   (inverse rDFT, /(E*E))

Pipeline: 4 groups of 4 batches, software-pipelined 2 groups deep so each
in-order engine queue always has ready work: tanh(g) runs under DFT(g+1),
inv/out(g) under DFT(g+2). Act-table loads are prefetched by dummy ops.
"""
import sys
sys.path.insert(0, '/opt/trn_rl_repo')
import numpy as np
from contextlib import ExitStack

import concourse.bacc as bacc
import concourse.mybir as mybir
import concourse.tile as tile
from concourse import masks
from concourse.bass_utils import run_bass_kernel_spmd

F32 = mybir.dt.float32
F16 = mybir.dt.float16
AF = mybir.ActivationFunctionType
ALU = mybir.AluOpType

B, E, H, L = 16, 64, 8, 2048
M = 64                      # modes
NCH = 16                    # l-chunks of 128
GRP = 4                     # batches per pipeline group
S_W = 4096.0                # weight prescale (2^12)
S_X = 2.0 ** -18            # X downcast scale (keeps Xs out of fp16 subnormals)
S_OUT = 2.0 ** 14           # 2^-6 remainder * 2^20 fp16-range boost
S_HOST = 2.0 ** -21         # host-side unscale (incl. 1/2 for doubled k)
S_C = 2.0 ** -11            # folded into Cinv (1/L)
# S_W * S_X * S_C * S_OUT = 2^12 * 2^-18 * 2^-11 * 2^-6 = 2^-23 = 1/(L*E*E)

INV2PI = float(1.0 / (2 * np.pi))
MAGIC = 12582912.0          # 1.5 * 2^23 round-to-int magic
C1 = 6.28125                # Cody-Waite 2pi hi (exact in 9 bits)
C2W = float(2 * np.pi - 6.28125)

_CACHE = {}
_DEBUG = False
_MARKS = []  # (label, "I-<n>") build-order markers for trace attribution

# Route Tanh AND Sin to the one act-func set that holds both
# (silu_and_others), so the whole kernel needs a single 1.3us table load
# instead of two per tanh block.
_orig_gat = bacc.get_activation_tables


def _gat_silu_only(arch):
    tables = _orig_gat(arch)
    for name, funcs in tables.items():
        if name != 'silu_and_others':
            funcs.discard(AF.Tanh)
            funcs.discard(AF.Sin)
    return tables


bacc.get_activation_tables = _gat_silu_only


def _dft_consts():
    l = np.arange(L)[:, None]
    m = np.arange(M)[None, :]
    ang = 2.0 * np.pi * l * m / L
    c2 = np.concatenate([np.cos(ang), -np.sin(ang)], axis=1)   # [L, 128]
    c2hi = c2.astype(np.float16)
    c2lo = (c2 - c2hi.astype(np.float64)).astype(np.float16)
    c2hi = c2hi.reshape(128, NCH, 128)
    c2lo = c2lo.reshape(128, NCH, 128)

    x = np.arange(M)[:, None]
    lr = np.arange(L)[None, :]
    ang2 = 2.0 * np.pi * x * lr / L
    dup = np.where(np.arange(M) == 0, 1.0, 2.0)[:, None]
    cr = dup * np.cos(ang2) * (S_C * S_OUT)
    ci = -dup * np.sin(ang2) * (S_C * S_OUT)
    ci[0, :] = 0.0
    cinv = np.concatenate([cr, ci], axis=0).astype(np.float16)  # [128, L]
    return c2hi, c2lo, cinv


def _tanh_block(nc, tb, e1, pack_re, pack_im):
    """Complex tanh of e1 = [64, re 0:256 | im 256:512] psum; writes the
    e2 rhs packs: pack_re=[2t/D ; -Im], pack_im=[Im ; 2t/D] as fp16.

    The serial chain lives on DVE (in-order, no sem hops); Act only does
    the three transcendentals; Pool takes off-critical-path scalar ops.
    Dummy act ops prefetch the tanh/trig tables off the critical path.
    """
    W = 256
    ps_re = e1[:, 0:W]
    ps_im = e1[:, W:2 * W]
    t = tb.tile([64, W], F32, tag="t")
    # k (and hence e1) is doubled host-side: tanh(re) = Tanh(ps_re * 0.5)
    nc.scalar.activation(t[:], ps_re, AF.Tanh, scale=0.5)
    # range reduction of z = ps_im mod 2pi (Cody-Waite); ps_im is already 2*Im
    u1 = tb.tile([64, W], F32, tag="u1")
    nc.vector.tensor_scalar(u1[:], ps_im, INV2PI, MAGIC,
                            ALU.mult, ALU.add)
    w1 = tb.tile([64, W], F32, tag="w1")
    nc.vector.tensor_scalar(w1[:], u1[:], MAGIC, C1, ALU.subtract, ALU.mult)
    w2 = tb.tile([64, W], F32, tag="w2")
    nc.gpsimd.tensor_scalar(w2[:], u1[:], MAGIC, C2W, ALU.subtract, ALU.mult)
    zr0 = tb.tile([64, W], F32, tag="zr0")
    nc.vector.tensor_sub(zr0[:], ps_im, w1[:])
    zr = tb.tile([64, W], F32, tag="zr")
    nc.vector.tensor_sub(zr[:], zr0[:], w2[:])
    s2 = tb.tile([64, W], F32, tag="s2")
    nc.scalar.activation(s2[:], zr[:], AF.Sin)
    sh = tb.tile([64, W], F32, tag="sh")
    nc.scalar.activation(sh[:], zr[:], AF.Sin, scale=0.5)
    t2 = tb.tile([64, W], F32, tag="t2")
    nc.vector.tensor_mul(t2[:], t[:], t[:])
    u = tb.tile([64, W], F32, tag="u")
    nc.gpsimd.tensor_scalar(u[:], t2[:], -1.0, 1.0, ALU.mult, ALU.add)
    sh2 = tb.tile([64, W], F32, tag="sh2")
    nc.vector.tensor_mul(sh2[:], sh[:], sh[:])
    # D/2 = 1 - u*sin^2(zr/2)  (exactly (2 - u + cos(zr)*u)/2); r = 2/D
    su = tb.tile([64, W], F32, tag="su")
    nc.vector.tensor_mul(su[:], s2[:], u[:])
    srh = tb.tile([64, W], F32, tag="srh")
    nc.gpsimd.tensor_scalar_mul(srh[:], su[:], 0.5)
    nsrh = tb.tile([64, W], F32, tag="nsrh")
    nc.gpsimd.tensor_scalar_mul(nsrh[:], su[:], -0.5)
    nsh = tb.tile([64, W], F32, tag="nsh")
    nc.vector.tensor_mul(nsh[:], u[:], sh2[:])
    dd2 = tb.tile([64, W], F32, tag="dd2")
    nc.vector.tensor_scalar(dd2[:], nsh[:], -1.0, 1.0, ALU.mult, ALU.add)
    r = tb.tile([64, W], F32, tag="r")
    nc.vector.reciprocal(r[:], dd2[:])
    # pack writes (fp16): pack_re=[2t/D ; -Im], pack_im=[Im ; 2t/D]
    nc.vector.tensor_mul(pack_re[0:64, :], t[:], r[:])
    nc.gpsimd.tensor_copy(pack_im[64:128, :], pack_re[0:64, :])
    nc.vector.tensor_mul(pack_im[0:64, :], srh[:], r[:])
    nc.gpsimd.tensor_mul(pack_re[64:128, :], nsrh[:], r[:])


def _build():
    nc = bacc.Bacc("TRN2", target_bir_lowering=False, debug=False)

    c2hi_np, c2lo_np, cinv_np = _dft_consts()
    C2H = nc.inline_tensor(np.ascontiguousarray(c2hi_np), name="C2H")
    C2L = nc.inline_tensor(np.ascontiguousarray(c2lo_np), name="C2L")
    CINV = nc.inline_tensor(np.ascontiguousarray(cinv_np), name="CINV")

    QK = nc.dram_tensor("qk", [B, 128, 2, NCH, 128], F16, kind="ExternalInput")
    WP = nc.dram_tensor("wp", [128, M, 128], F16, kind="ExternalInput")
    OUT = nc.dram_tensor("out", [B, E, L], F16, kind="ExternalOutput")
    DBG_E1 = nc.dram_tensor("dbg_e1", [4, 64, 512], F32, kind="ExternalOutput")
    DBG_PR = nc.dram_tensor("dbg_pr", [4, 128, 256], F16,
                            kind="ExternalOutput")
    DBG_PI = nc.dram_tensor("dbg_pi", [4, 128, 256], F16,
                            kind="ExternalOutput")
    DBG_VP = nc.dram_tensor("dbg_vp", [4, 128, 256], F16,
                            kind="ExternalOutput")
    DBG_XS = nc.dram_tensor("dbg_xs", [4, 128, 256], F16,
                            kind="ExternalOutput")
    DBG_FT = nc.dram_tensor("dbg_ft", [B, 2, 64, 128], F16,
                            kind="ExternalOutput")

    with tile.TileContext(nc) as tc, ExitStack() as ctx:
        cpool = ctx.enter_context(tc.tile_pool(name="consts", bufs=1))
        qk_pool = ctx.enter_context(tc.tile_pool(name="qk", bufs=10))
        ft_pool = ctx.enter_context(tc.tile_pool(name="ft", bufs=3))
        ktr_pool = ctx.enter_context(tc.tile_pool(name="ktr", bufs=16))
        tb_pool = ctx.enter_context(tc.tile_pool(name="tanh", bufs=2))
        st_pool = ctx.enter_context(tc.tile_pool(name="stage", bufs=3))
        # PSUM budget (8 banks): pf 2 + pe1 2 + pv 1 + pwo 2 + psm 1
        pf_pool = ctx.enter_context(
            tc.tile_pool(name="pf", bufs=2, space="PSUM"))
        pe1_pool = ctx.enter_context(
            tc.tile_pool(name="pe1", bufs=2, space="PSUM"))
        pv_pool = ctx.enter_context(
            tc.tile_pool(name="pv", bufs=1, space="PSUM"))
        pwo_pool = ctx.enter_context(
            tc.tile_pool(name="pwo", bufs=2, space="PSUM"))
        psm_pool = ctx.enter_context(
            tc.tile_pool(name="psm", bufs=1, space="PSUM"))

        c2h = cpool.tile([128, NCH, 128], F16)
        nc.sync.dma_start(c2h[:], C2H[:])
        c2l = cpool.tile([128, NCH, 128], F16)
        cinv = cpool.tile([128, L], F16)
        wp = cpool.tile([128, M, 128], F16)
        ident = cpool.tile([128, 128], F16)
        masks.make_identity(nc, ident[:])

        ktrs = [None] * B
        e1s = {}
        packs = {}
        vpacks = {}
        xss = {}
        lhss = {}

        def mark(lbl):
            _MARKS.append((lbl, nc.get_next_instruction_name()))

        qkts = [None] * B

        def emit_qk_dma(b):
            qkt = qk_pool.tile([128, 2, NCH, 128], F16, tag="qkt",
                               name="qkt")
            nc.sync.dma_start(qkt[:], QK[b])
            if b == 0:
                nc.sync.dma_start(c2l[:], C2L[:])
            elif b == 1:
                nc.sync.dma_start(wp[:], WP[:])
            elif b == 2:
                nc.sync.dma_start(cinv[:], CINV[:])
            qkts[b] = qkt

        def emit_dft_batch(g, j):
            b = g * GRP + j
            mark(f"dft({g},{j})")
            ctx_p = tc.high_priority(offset=1000000)
            ctx_p.__enter__()
            # ---------- forward DFT ----------
            qkt = qkts[b]
            qkh = qkt[:, 0]
            qkl = qkt[:, 1]

            ps_f = pf_pool.tile([128, 128], F32, tag="f")
            for n in range(NCH):
                nc.tensor.matmul(ps_f[:], qkh[:, n], c2h[:, n, :],
                                 start=(n == 0), stop=False)
                nc.tensor.matmul(ps_f[:], qkh[:, n], c2l[:, n, :],
                                 start=False, stop=False)
                nc.tensor.matmul(ps_f[:], qkl[:, n], c2h[:, n, :],
                                 start=False, stop=(n == NCH - 1))

            # ---------- hi/lo splits ----------
            # k-half staged to base-0 SBUF (TensorTensor inputs must share
            # a start partition); q-half splits read the psum directly
            ftqH = ft_pool.tile([64, 128], F16, tag="ftqH")
            nc.scalar.copy(ftqH[:], ps_f[0:64, :])
            ftqL = ft_pool.tile([64, 128], F16, tag="ftqL")
            nc.vector.tensor_sub(ftqL[:], ps_f[0:64, :], ftqH[:])
            ftqN = ft_pool.tile([64, 128], F16, tag="ftqN")
            nc.gpsimd.tensor_scalar_mul(ftqN[:, 0:64], ftqH[:, 64:128], -1.0)
            nc.gpsimd.tensor_scalar_mul(ftqN[:, 64:128], ftqL[:, 64:128],
                                        -1.0)
            kst = ft_pool.tile([64, 128], F32, tag="kst")
            nc.scalar.copy(kst[:], ps_f[64:128, :])
            ftkH = ft_pool.tile([64, 128], F16, tag="ftkH")
            nc.gpsimd.tensor_copy(ftkH[:], kst[:])
            ftkL = ft_pool.tile([64, 128], F16, tag="ftkL")
            nc.vector.tensor_sub(ftkL[:], kst[:], ftkH[:])
            if _DEBUG:
                nc.sync.dma_start(DBG_FT[b, 0], ftqH[:])
                nc.sync.dma_start(DBG_FT[b, 1], ftkH[:])

            # ---------- ktr = [KreT ; KimT] [128, 64] for e2 ----------
            ps_t = psm_pool.tile([128, 64], F16, tag="sm", name="ps_t")
            nc.tensor.transpose(ps_t[:], ftkH[:], ident[0:64, 0:64])
            ktr = ktr_pool.tile([128, 64], F16, tag="ktr")
            nc.vector.tensor_copy(ktr[:], ps_t[:])
            ktrs[b] = ktr

            # ---------- e1 ----------
            e1 = e1s[g]
            col = j * 64
            out_re = e1[:, col:col + 64]
            out_im = e1[:, 256 + col:256 + col + 64]
            KreH, KimH = ftkH[:, 0:64], ftkH[:, 64:128]
            KreL, KimL = ftkL[:, 0:64], ftkL[:, 64:128]
            QreH, QimH = ftqH[:, 0:64], ftqH[:, 64:128]
            QreL, QimL = ftqL[:, 0:64], ftqL[:, 64:128]
            nQimH, nQimL = ftqN[:, 0:64], ftqN[:, 64:128]
            # one pending accumulation stream per psum bank at a time: a
            # start=True wipes the bank's un-stopped partials on hardware,
            # so re must fully commit (stop) before im starts
            nc.tensor.matmul(out_re, KreH, QreH, start=True, stop=False)
            nc.tensor.matmul(out_re, KreH, QreL, start=False, stop=False)
            nc.tensor.matmul(out_re, KimH, nQimH, start=False, stop=False)
            nc.tensor.matmul(out_re, KimH, nQimL, start=False, stop=False)
            nc.tensor.matmul(out_re, KreL, QreH, start=False, stop=False)
            nc.tensor.matmul(out_re, KimL, nQimH, start=False, stop=True)
            nc.tensor.matmul(out_im, KreH, QimH, start=True, stop=False)
            nc.tensor.matmul(out_im, KreH, QimL, start=False, stop=False)
            nc.tensor.matmul(out_im, KimH, QreH, start=False, stop=False)
            nc.tensor.matmul(out_im, KimH, QreL, start=False, stop=False)
            nc.tensor.matmul(out_im, KreL, QimH, start=False, stop=False)
            nc.tensor.matmul(out_im, KimL, QreH, start=False, stop=True)
            ctx_p.__exit__(None, None, None)

        def emit_tanh(g):
            mark(f"tanh({g})")
            ctx_p = tc.high_priority(offset=800000)
            ctx_p.__enter__()
            if _DEBUG:
                e1st = tb_pool.tile([64, 512], F32, tag="e1st", name="e1st")
                nc.scalar.copy(e1st[:], e1s[g][:])
                nc.sync.dma_start(DBG_E1[g], e1st[:])
            pack_re = tb_pool.tile([128, 256], F16, tag="pack_re")
            pack_im = tb_pool.tile([128, 256], F16, tag="pack_im")
            _tanh_block(nc, tb_pool, e1s[g], pack_re, pack_im)
            packs[g] = (pack_re, pack_im)
            ctx_p.__exit__(None, None, None)
            if _DEBUG:
                nc.sync.dma_start(DBG_PR[g], pack_re[:])
                nc.sync.dma_start(DBG_PI[g], pack_im[:])

        def emit_e2(g):
            mark(f"e2({g})")
            ctx_p = tc.high_priority(offset=600000)
            ctx_p.__enter__()
            pack_re, pack_im = packs[g]
            pv = pv_pool.tile([64, 512], F32, tag="v")
            for j in range(GRP):
                b = g * GRP + j
                col = j * 64
                nc.tensor.matmul(pv[:, col:col + 64], ktrs[b][:],
                                 pack_re[:, col:col + 64],
                                 start=True, stop=True)
                nc.tensor.matmul(pv[:, 256 + col:256 + col + 64], ktrs[b][:],
                                 pack_im[:, col:col + 64],
                                 start=True, stop=True)

            # vpack [e_re|e_im, b*64+x]
            vpack = st_pool.tile([128, 256], F16, tag="vpack")
            nc.scalar.copy(vpack[0:64, :], pv[:, 0:256])
            nc.vector.tensor_copy(vpack[64:128, :], pv[:, 256:512])
            vpacks[g] = vpack
            ctx_p.__exit__(None, None, None)
            if _DEBUG:
                nc.sync.dma_start(DBG_VP[g], vpack[:])

        def emit_wmix(g):
            mark(f"wmix({g})")
            ctx_p = tc.high_priority(offset=500000)
            ctx_p.__enter__()
            vpack = vpacks[g]
            ps_w = pwo_pool.tile([128, 256], F32, tag="wo")
            vp3 = vpack[:].rearrange("p (b x) -> p b x", x=64)
            for x in range(M):
                nc.tensor.matmul(ps_w[:, x * GRP:(x + 1) * GRP], wp[:, x, :],
                                 vp3[:, :, x], start=True, stop=True)

            xs = st_pool.tile([128, 256], F16, tag="xs")
            nc.scalar.activation(xs[:], ps_w[:], AF.Copy, bias=0.0, scale=S_X)
            xss[g] = xs
            ctx_p.__exit__(None, None, None)
            if _DEBUG:
                nc.sync.dma_start(DBG_XS[g], xs[:])

        def emit_xt(g):
            mark(f"xt({g})")
            ctx_p = tc.high_priority(offset=400000)
            ctx_p.__enter__()
            xs3 = xss[g][:].rearrange("p (x b) -> p x b", b=GRP)
            pair_lhs = []
            for bp in range(GRP // 2):
                lhs = st_pool.tile([128, 128], F16, tag="lhs_inv")
                for j in range(2):
                    bb = 2 * bp + j
                    ps_xt = pv_pool.tile([128, 64], F16, tag="v",
                                         name="ps_xt")
                    nc.tensor.transpose(ps_xt[0:64, :], xs3[0:64, :, bb],
                                        ident[0:64, 0:64])
                    nc.tensor.transpose(ps_xt[64:128, :], xs3[64:128, :, bb],
                                        ident[64:128, 64:128])
                    if j == 0:
                        nc.scalar.copy(lhs[:, 0:64], ps_xt[:])
                    else:
                        nc.vector.tensor_copy(lhs[:, 64:128], ps_xt[:])
                pair_lhs.append(lhs)
            lhss[g] = pair_lhs
            ctx_p.__exit__(None, None, None)

        def emit_inv_pair(g, bp):
            mark(f"inv({g},{bp})")
            lhs = lhss[g][bp]
            bo = g * GRP + 2 * bp
            ov = OUT[bo:bo + 2, :, :].rearrange("b o l -> (b o) l")
            ot = st_pool.tile([128, 2048], F16, tag="ostage")
            for c in range(4):
                ps_o = pwo_pool.tile([128, 512], F32, tag="wo")
                nc.tensor.matmul(ps_o[:], lhs[:],
                                 cinv[:, c * 512:(c + 1) * 512],
                                 start=True, stop=True)
                if c % 2 == 0:
                    nc.scalar.copy(ot[:, c * 512:(c + 1) * 512], ps_o[:])
                else:
                    nc.vector.tensor_copy(ot[:, c * 512:(c + 1) * 512],
                                          ps_o[:])
                # chunked store rides the otherwise-idle SP sequencer; all
                # qk loads were emitted in the prologue so no HOL risk
                nc.sync.dma_start(ov[:, c * 512:(c + 1) * 512],
                                  ot[:, c * 512:(c + 1) * 512])

        # PE warmup: dummy matmuls while the first QK load is in flight, so
        # the tensor engine is past its p-state ramp when real work arrives.
        wu = pwo_pool.tile([128, 512], F32, tag="wo", name="wu")
        for i in range(20):
            nc.tensor.matmul(wu[:, 0:128], ident[:], ident[:],
                             start=(i == 0), stop=(i == 19))
        # Act-table warmup: the single tanh+sin set loads once, off-path
        wud = tb_pool.tile([64, 1], F32, tag="dummy", name="wud")
        nc.scalar.activation(wud[:], ident[0:64, 0:1], AF.Tanh)

        # Software pipeline, 2 groups deep: group g's stages are spread
        # under DFT(g+1) (tanh/e2/wmix/xt) and DFT(g+2) (inv+out) so the
        # in-order engine queues never head-of-line-block the next group.
        for b in range(B):
            emit_qk_dma(b)

        NG = B // GRP
        stages = {}

        def at(slot, fn):
            stages.setdefault(slot, []).append(fn)

        for g in range(NG):
            at((g + 1, 0), lambda g=g: emit_tanh(g))
            at((g + 1, 3), lambda g=g: emit_e2(g))
        for g in range(NG):
            at((g + 2, 0), lambda g=g: emit_wmix(g))
            at((g + 2, 1), lambda g=g: emit_xt(g))
            at((g + 2, 2), lambda g=g: emit_inv_pair(g, 0))
            at((g + 2, 3), lambda g=g: emit_inv_pair(g, 1))

        for g in range(NG + 2):
            if g < NG:
                e1s[g] = pe1_pool.tile([64, 512], F32, tag="e1", name="e1")
            for fn in stages.get((g, 0), []):
                fn()
            for j in range(GRP):
                if g < NG:
                    emit_dft_batch(g, j)
                if j > 0:
                    for fn in stages.get((g, j), []):
                        fn()

    nc.compile()
    return nc


def _host_pack(q, k, w_re, w_im):
    in_maps = []
    for h in range(H):
        qT = np.ascontiguousarray(q[:, :, h, :].transpose(0, 2, 1))
        kT = np.ascontiguousarray(k[:, :, h, :].transpose(0, 2, 1))
        qk = np.concatenate([qT, 2.0 * kT], axis=2)    # [B, L, 128]
        qkh = qk.astype(np.float16)
        qkl = (qk - qkh.astype(np.float32)).astype(np.float16)
        qkhl = np.stack([qkh.reshape(B, 128, NCH, 128),
                         qkl.reshape(B, 128, NCH, 128)], axis=2)

        wre = (w_re[h] * S_W).astype(np.float32)       # [E, E(o), M]
        wim = (w_im[h] * S_W).astype(np.float32)
        wp = np.empty((128, M, 128), dtype=np.float16)
        wp[0:64, :, 0:64] = wre.transpose(0, 2, 1)
        wp[64:128, :, 0:64] = -wim.transpose(0, 2, 1)
        wp[0:64, :, 64:128] = wim.transpose(0, 2, 1)
        wp[64:128, :, 64:128] = wre.transpose(0, 2, 1)
        in_maps.append({
            "qk": np.ascontiguousarray(qkhl),
            "wp": wp,
        })
    return in_maps


def kernel(q, k, v, w_re, w_im, _trace=False):
    q = np.asarray(q, dtype=np.float32)
    k = np.asarray(k, dtype=np.float32)
    w_re = np.asarray(w_re, dtype=np.float32)
    w_im = np.asarray(w_im, dtype=np.float32)

    if "nc" not in _CACHE:
        _CACHE["nc"] = _build()
    nc = _CACHE["nc"]

    in_maps = _host_pack(q, k, w_re, w_im)
    res = None
    for attempt in range(3):
        try:
            res = run_bass_kernel_spmd(nc, in_maps, list(range(H)),
                                       trace=_trace)
            break
        except Exception:
            if attempt == 2:
                raise
            import time as _time
            _time.sleep(5.0)
    out = np.stack([res.results[h]["out"].astype(np.float32) * S_HOST
                    for h in range(H)], axis=2)
    _CACHE["last_result"] = res
    return np.ascontiguousarray(out)  # [B, E, H, L]


# revision 50
# speedup vs baseline: 1.2495x; 1.0162x over previous
"""FourierCrossAttention Trainium2 kernel.

Sharding: one head per NeuronCore (H=8, n_cores=8); each core processes all
B=16 batches for its head.

Math (per (b,h)):
  ftq = qT @ [cos | -sin]           (DFT, first 64 modes; fp16 hi/lo split)
  ftk = kT @ [cos | -sin]
  xy[y,x]   = sum_e ftk[e,y]*ftq[e,x]      (complex)
  A = tanh(xy)                             (complex tanh, stable form)
  v[e,x]    = sum_y ftk[e,y]*A[x,y]        (complex)
  X[o,x]    = sum_e v[e,x]*W[e,o,x]        (complex, per-head weights)
  out[o,l]  = sum_x Re(X)*cr[x,l] + Im(X)*ci[x,l]   (inverse rDFT, /(E*E))

Pipeline: 4 groups of 4 batches, software-pipelined 2 groups deep so each
in-order engine queue always has ready work: tanh(g) runs under DFT(g+1),
inv/out(g) under DFT(g+2). Act-table loads are prefetched by dummy ops.
"""
import sys
sys.path.insert(0, '/opt/trn_rl_repo')
import numpy as np
from contextlib import ExitStack

import concourse.bacc as bacc
import concourse.mybir as mybir
import concourse.tile as tile
from concourse import masks
from concourse.bass_utils import run_bass_kernel_spmd

F32 = mybir.dt.float32
F16 = mybir.dt.float16
AF = mybir.ActivationFunctionType
ALU = mybir.AluOpType

B, E, H, L = 16, 64, 8, 2048
M = 64                      # modes
NCH = 16                    # l-chunks of 128
GROUPS = [4, 4, 4, 4]    # batches per pipeline group; small tail groups
GSTART = [sum(GROUPS[:i]) for i in range(len(GROUPS))]
S_W = 4096.0                # weight prescale (2^12)
S_X = 2.0 ** -18            # X downcast scale (keeps Xs out of fp16 subnormals)
S_OUT = 2.0 ** 14           # 2^-6 remainder * 2^20 fp16-range boost
S_HOST = 2.0 ** -21         # host-side unscale (incl. 1/2 for doubled k)
S_C = 2.0 ** -11            # folded into Cinv (1/L)
# S_W * S_X * S_C * S_OUT = 2^12 * 2^-18 * 2^-11 * 2^-6 = 2^-23 = 1/(L*E*E)

INV2PI = float(1.0 / (2 * np.pi))
MAGIC = 12582912.0          # 1.5 * 2^23 round-to-int magic
C1 = 6.28125                # Cody-Waite 2pi hi (exact in 9 bits)
C2W = float(2 * np.pi - 6.28125)

_CACHE = {}
_DEBUG = False
_MARKS = []  # (label, "I-<n>") build-order markers for trace attribution

# Route Tanh AND Sin to the one act-func set that holds both
# (silu_and_others), so the whole kernel needs a single 1.3us table load
# instead of two per tanh block.
_orig_gat = bacc.get_activation_tables


def _gat_silu_only(arch):
    tables = _orig_gat(arch)
    for name, funcs in tables.items():
        if name != 'silu_and_others':
            funcs.discard(AF.Tanh)
            funcs.discard(AF.Sin)
    return tables


bacc.get_activation_tables = _gat_silu_only


def _dft_consts():
    l = np.arange(L)[:, None]
    m = np.arange(M)[None, :]
    ang = 2.0 * np.pi * l * m / L
    c2 = np.concatenate([np.cos(ang), -np.sin(ang)], axis=1)   # [L, 128]
    c2hi = c2.astype(np.float16)
    c2lo = (c2 - c2hi.astype(np.float64)).astype(np.float16)
    c2hi = c2hi.reshape(128, NCH, 128)
    c2lo = c2lo.reshape(128, NCH, 128)

    x = np.arange(M)[:, None]
    lr = np.arange(L)[None, :]
    ang2 = 2.0 * np.pi * x * lr / L
    dup = np.where(np.arange(M) == 0, 1.0, 2.0)[:, None]
    cr = dup * np.cos(ang2) * (S_C * S_OUT)
    ci = -dup * np.sin(ang2) * (S_C * S_OUT)
    ci[0, :] = 0.0
    cinv = np.concatenate([cr, ci], axis=0).astype(np.float16)  # [128, L]
    return c2hi, c2lo, cinv


def _tanh_block(nc, tb, e1, pack_re, pack_im, W):
    """Complex tanh of e1 = [64, re 0:W | im W:2W] psum; writes the
    e2 rhs packs: pack_re=[2t/D ; -Im], pack_im=[Im ; 2t/D] as fp16.

    The serial chain lives on DVE (in-order, no sem hops); Act only does
    the three transcendentals; Pool takes off-critical-path scalar ops.
    """
    ps_re = e1[:, 0:W]
    ps_im = e1[:, W:2 * W]
    t = tb.tile([64, W], F32, tag="t")
    # k (and hence e1) is doubled host-side: tanh(re) = Tanh(ps_re * 0.5)
    nc.scalar.activation(t[:], ps_re, AF.Tanh, scale=0.5)
    # range reduction of z = ps_im mod 2pi (Cody-Waite); ps_im is already 2*Im
    u1 = tb.tile([64, W], F32, tag="u1")
    nc.vector.tensor_scalar(u1[:], ps_im, INV2PI, MAGIC,
                            ALU.mult, ALU.add)
    w1 = tb.tile([64, W], F32, tag="w1")
    nc.vector.tensor_scalar(w1[:], u1[:], MAGIC, C1, ALU.subtract, ALU.mult)
    w2 = tb.tile([64, W], F32, tag="w2")
    nc.gpsimd.tensor_scalar(w2[:], u1[:], MAGIC, C2W, ALU.subtract, ALU.mult)
    zr0 = tb.tile([64, W], F32, tag="zr0")
    nc.vector.tensor_sub(zr0[:], ps_im, w1[:])
    zr = tb.tile([64, W], F32, tag="zr")
    nc.vector.tensor_sub(zr[:], zr0[:], w2[:])
    s2 = tb.tile([64, W], F32, tag="s2")
    nc.scalar.activation(s2[:], zr[:], AF.Sin)
    sh = tb.tile([64, W], F32, tag="sh")
    nc.scalar.activation(sh[:], zr[:], AF.Sin, scale=0.5)
    t2 = tb.tile([64, W], F32, tag="t2")
    nc.vector.tensor_mul(t2[:], t[:], t[:])
    u = tb.tile([64, W], F32, tag="u")
    nc.gpsimd.tensor_scalar(u[:], t2[:], -1.0, 1.0, ALU.mult, ALU.add)
    sh2 = tb.tile([64, W], F32, tag="sh2")
    nc.vector.tensor_mul(sh2[:], sh[:], sh[:])
    # D/2 = 1 - u*sin^2(zr/2)  (exactly (2 - u + cos(zr)*u)/2); r = 2/D
    su = tb.tile([64, W], F32, tag="su")
    nc.vector.tensor_mul(su[:], s2[:], u[:])
    srh = tb.tile([64, W], F32, tag="srh")
    nc.gpsimd.tensor_scalar_mul(srh[:], su[:], 0.5)
    nsrh = tb.tile([64, W], F32, tag="nsrh")
    nc.gpsimd.tensor_scalar_mul(nsrh[:], su[:], -0.5)
    nsh = tb.tile([64, W], F32, tag="nsh")
    nc.vector.tensor_mul(nsh[:], u[:], sh2[:])
    dd2 = tb.tile([64, W], F32, tag="dd2")
    nc.vector.tensor_scalar(dd2[:], nsh[:], -1.0, 1.0, ALU.mult, ALU.add)
    r = tb.tile([64, W], F32, tag="r")
    nc.vector.reciprocal(r[:], dd2[:])
    # pack writes (fp16): pack_re=[2t/D ; -Im], pack_im=[Im ; 2t/D]
    nc.vector.tensor_mul(pack_re[0:64, :], t[:], r[:])
    nc.gpsimd.tensor_copy(pack_im[64:128, :], pack_re[0:64, :])
    nc.vector.tensor_mul(pack_im[0:64, :], srh[:], r[:])
    nc.gpsimd.tensor_mul(pack_re[64:128, :], nsrh[:], r[:])


def _build():
    nc = bacc.Bacc("TRN2", target_bir_lowering=False, debug=False)

    c2hi_np, c2lo_np, cinv_np = _dft_consts()
    C2H = nc.inline_tensor(np.ascontiguousarray(c2hi_np), name="C2H")
    C2L = nc.inline_tensor(np.ascontiguousarray(c2lo_np), name="C2L")
    CINV = nc.inline_tensor(np.ascontiguousarray(cinv_np), name="CINV")

    QK = nc.dram_tensor("qk", [B, 128, 2, NCH, 128], F16, kind="ExternalInput")
    WP = nc.dram_tensor("wp", [128, M, 128], F16, kind="ExternalInput")
    OUT = nc.dram_tensor("out", [B, E, L], F16, kind="ExternalOutput")
    DBG_E1 = nc.dram_tensor("dbg_e1", [4, 64, 512], F32, kind="ExternalOutput")
    DBG_PR = nc.dram_tensor("dbg_pr", [4, 128, 256], F16,
                            kind="ExternalOutput")
    DBG_PI = nc.dram_tensor("dbg_pi", [4, 128, 256], F16,
                            kind="ExternalOutput")
    DBG_VP = nc.dram_tensor("dbg_vp", [4, 128, 256], F16,
                            kind="ExternalOutput")
    DBG_XS = nc.dram_tensor("dbg_xs", [4, 128, 256], F16,
                            kind="ExternalOutput")
    DBG_FT = nc.dram_tensor("dbg_ft", [B, 2, 64, 128], F16,
                            kind="ExternalOutput")

    with tile.TileContext(nc) as tc, ExitStack() as ctx:
        cpool = ctx.enter_context(tc.tile_pool(name="consts", bufs=1))
        qk_pool = ctx.enter_context(tc.tile_pool(name="qk", bufs=10))
        ft_pool = ctx.enter_context(tc.tile_pool(name="ft", bufs=3))
        ktr_pool = ctx.enter_context(tc.tile_pool(name="ktr", bufs=16))
        tb_pool = ctx.enter_context(tc.tile_pool(name="tanh", bufs=2))
        st_pool = ctx.enter_context(tc.tile_pool(name="stage", bufs=3))
        # PSUM budget (8 banks): pf 2 + pe1 2 + pv 1 + pwo 2 + psm 1
        pf_pool = ctx.enter_context(
            tc.tile_pool(name="pf", bufs=2, space="PSUM"))
        pe1_pool = ctx.enter_context(
            tc.tile_pool(name="pe1", bufs=2, space="PSUM"))
        pv_pool = ctx.enter_context(
            tc.tile_pool(name="pv", bufs=1, space="PSUM"))
        pwo_pool = ctx.enter_context(
            tc.tile_pool(name="pwo", bufs=2, space="PSUM"))
        psm_pool = ctx.enter_context(
            tc.tile_pool(name="psm", bufs=1, space="PSUM"))

        c2h = cpool.tile([128, NCH, 128], F16)
        nc.sync.dma_start(c2h[:], C2H[:])
        c2l = cpool.tile([128, NCH, 128], F16)
        cinv = cpool.tile([128, L], F16)
        wp = cpool.tile([128, M, 128], F16)
        ident = cpool.tile([128, 128], F16)
        masks.make_identity(nc, ident[:])

        ktrs = [None] * B
        e1s = {}
        packs = {}
        vpacks = {}
        xss = {}
        lhss = {}

        def mark(lbl):
            _MARKS.append((lbl, nc.get_next_instruction_name()))

        qkts = [None] * B

        def emit_qk_dma(b):
            qkt = qk_pool.tile([128, 2, NCH, 128], F16, tag="qkt",
                               name="qkt")
            nc.sync.dma_start(qkt[:], QK[b])
            if b == 0:
                nc.sync.dma_start(c2l[:], C2L[:])
            elif b == 1:
                nc.sync.dma_start(wp[:], WP[:])
            elif b == 2:
                nc.sync.dma_start(cinv[:], CINV[:])
            qkts[b] = qkt

        def emit_dft_batch(g, j):
            b = GSTART[g] + j
            W = GROUPS[g] * 64
            mark(f"dft({g},{j})")
            ctx_p = tc.high_priority(offset=1000000)
            ctx_p.__enter__()
            # ---------- forward DFT ----------
            qkt = qkts[b]
            qkh = qkt[:, 0]
            qkl = qkt[:, 1]

            ps_f = pf_pool.tile([128, 128], F32, tag="f")
            for n in range(NCH):
                nc.tensor.matmul(ps_f[:], qkh[:, n], c2h[:, n, :],
                                 start=(n == 0), stop=False)
                nc.tensor.matmul(ps_f[:], qkh[:, n], c2l[:, n, :],
                                 start=False, stop=False)
                nc.tensor.matmul(ps_f[:], qkl[:, n], c2h[:, n, :],
                                 start=False, stop=(n == NCH - 1))

            # ---------- hi/lo splits ----------
            # k-half staged to base-0 SBUF (TensorTensor inputs must share
            # a start partition); q-half splits read the psum directly
            ftqH = ft_pool.tile([64, 128], F16, tag="ftqH")
            nc.scalar.copy(ftqH[:], ps_f[0:64, :])
            ftqL = ft_pool.tile([64, 128], F16, tag="ftqL")
            nc.vector.tensor_sub(ftqL[:], ps_f[0:64, :], ftqH[:])
            ftqN = ft_pool.tile([64, 128], F16, tag="ftqN")
            nc.gpsimd.tensor_scalar_mul(ftqN[:, 0:64], ftqH[:, 64:128], -1.0)
            nc.gpsimd.tensor_scalar_mul(ftqN[:, 64:128], ftqL[:, 64:128],
                                        -1.0)
            kst = ft_pool.tile([64, 128], F32, tag="kst")
            nc.scalar.copy(kst[:], ps_f[64:128, :])
            ftkH = ft_pool.tile([64, 128], F16, tag="ftkH")
            nc.gpsimd.tensor_copy(ftkH[:], kst[:])
            ftkL = ft_pool.tile([64, 128], F16, tag="ftkL")
            nc.vector.tensor_sub(ftkL[:], kst[:], ftkH[:])
            if _DEBUG:
                nc.sync.dma_start(DBG_FT[b, 0], ftqH[:])
                nc.sync.dma_start(DBG_FT[b, 1], ftkH[:])

            # ---------- ktr = [KreT ; KimT] [128, 64] for e2 ----------
            ps_t = psm_pool.tile([128, 64], F16, tag="sm", name="ps_t")
            nc.tensor.transpose(ps_t[:], ftkH[:], ident[0:64, 0:64])
            ktr = ktr_pool.tile([128, 64], F16, tag="ktr")
            nc.vector.tensor_copy(ktr[:], ps_t[:])
            ktrs[b] = ktr

            # ---------- e1 ----------
            e1 = e1s[g]
            col = j * 64
            out_re = e1[:, col:col + 64]
            out_im = e1[:, W + col:W + col + 64]
            KreH, KimH = ftkH[:, 0:64], ftkH[:, 64:128]
            KreL, KimL = ftkL[:, 0:64], ftkL[:, 64:128]
            QreH, QimH = ftqH[:, 0:64], ftqH[:, 64:128]
            QreL, QimL = ftqL[:, 0:64], ftqL[:, 64:128]
            nQimH, nQimL = ftqN[:, 0:64], ftqN[:, 64:128]
            # one pending accumulation stream per psum bank at a time: a
            # start=True wipes the bank's un-stopped partials on hardware,
            # so re must fully commit (stop) before im starts
            nc.tensor.matmul(out_re, KreH, QreH, start=True, stop=False)
            nc.tensor.matmul(out_re, KreH, QreL, start=False, stop=False)
            nc.tensor.matmul(out_re, KimH, nQimH, start=False, stop=False)
            nc.tensor.matmul(out_re, KimH, nQimL, start=False, stop=False)
            nc.tensor.matmul(out_re, KreL, QreH, start=False, stop=False)
            nc.tensor.matmul(out_re, KimL, nQimH, start=False, stop=True)
            nc.tensor.matmul(out_im, KreH, QimH, start=True, stop=False)
            nc.tensor.matmul(out_im, KreH, QimL, start=False, stop=False)
            nc.tensor.matmul(out_im, KimH, QreH, start=False, stop=False)
            nc.tensor.matmul(out_im, KimH, QreL, start=False, stop=False)
            nc.tensor.matmul(out_im, KreL, QimH, start=False, stop=False)
            nc.tensor.matmul(out_im, KimL, QreH, start=False, stop=True)
            ctx_p.__exit__(None, None, None)

        def emit_tanh(g):
            mark(f"tanh({g})")
            ctx_p = tc.high_priority(offset=800000)
            ctx_p.__enter__()
            W = GROUPS[g] * 64
            if _DEBUG and GROUPS[g] == 4:
                e1st = tb_pool.tile([64, 512], F32, tag="e1st", name="e1st")
                nc.scalar.copy(e1st[:], e1s[g][:])
                nc.sync.dma_start(DBG_E1[g], e1st[:])
            pack_re = tb_pool.tile([128, W], F16, tag="pack_re",
                                   name="pack_re")
            pack_im = tb_pool.tile([128, W], F16, tag="pack_im",
                                   name="pack_im")
            _tanh_block(nc, tb_pool, e1s[g], pack_re, pack_im, W)
            packs[g] = (pack_re, pack_im)
            ctx_p.__exit__(None, None, None)
            if _DEBUG and GROUPS[g] == 4:
                nc.sync.dma_start(DBG_PR[g], pack_re[:])
                nc.sync.dma_start(DBG_PI[g], pack_im[:])

        def emit_e2(g):
            mark(f"e2({g})")
            ctx_p = tc.high_priority(offset=600000)
            ctx_p.__enter__()
            W = GROUPS[g] * 64
            pack_re, pack_im = packs[g]
            pv = pv_pool.tile([64, 2 * W], F32, tag="v", name="pv")
            for j in range(GROUPS[g]):
                b = GSTART[g] + j
                col = j * 64
                nc.tensor.matmul(pv[:, col:col + 64], ktrs[b][:],
                                 pack_re[:, col:col + 64],
                                 start=True, stop=True)
                nc.tensor.matmul(pv[:, W + col:W + col + 64], ktrs[b][:],
                                 pack_im[:, col:col + 64],
                                 start=True, stop=True)

            # vpack [e_re|e_im, b*64+x]
            vpack = st_pool.tile([128, W], F16, tag="vpack", name="vpack")
            nc.scalar.copy(vpack[0:64, :], pv[:, 0:W])
            nc.vector.tensor_copy(vpack[64:128, :], pv[:, W:2 * W])
            vpacks[g] = vpack
            ctx_p.__exit__(None, None, None)
            if _DEBUG and GROUPS[g] == 4:
                nc.sync.dma_start(DBG_VP[g], vpack[:])

        def emit_wmix(g):
            mark(f"wmix({g})")
            ctx_p = tc.high_priority(offset=500000)
            ctx_p.__enter__()
            G = GROUPS[g]
            vpack = vpacks[g]
            ps_w = pwo_pool.tile([128, G * 64], F32, tag="wo", name="ps_w")
            vp3 = vpack[:].rearrange("p (b x) -> p b x", x=64)
            for x in range(M):
                nc.tensor.matmul(ps_w[:, x * G:(x + 1) * G], wp[:, x, :],
                                 vp3[:, :, x], start=True, stop=True)

            xs = st_pool.tile([128, G * 64], F16, tag="xs", name="xs")
            nc.scalar.activation(xs[:], ps_w[:], AF.Copy, bias=0.0, scale=S_X)
            xss[g] = xs
            ctx_p.__exit__(None, None, None)
            if _DEBUG and GROUPS[g] == 4:
                nc.sync.dma_start(DBG_XS[g], xs[:])

        def emit_xt(g):
            mark(f"xt({g})")
            ctx_p = tc.high_priority(offset=400000)
            ctx_p.__enter__()
            xs3 = xss[g][:].rearrange("p (x b) -> p x b", b=GROUPS[g])
            pair_lhs = []
            for bp in range(GROUPS[g] // 2):
                lhs = st_pool.tile([128, 128], F16, tag="lhs_inv")
                for j in range(2):
                    bb = 2 * bp + j
                    ps_xt = pv_pool.tile([128, 64], F16, tag="v",
                                         name="ps_xt")
                    nc.tensor.transpose(ps_xt[0:64, :], xs3[0:64, :, bb],
                                        ident[0:64, 0:64])
                    nc.tensor.transpose(ps_xt[64:128, :], xs3[64:128, :, bb],
                                        ident[64:128, 64:128])
                    if j == 0:
                        nc.scalar.copy(lhs[:, 0:64], ps_xt[:])
                    else:
                        nc.vector.tensor_copy(lhs[:, 64:128], ps_xt[:])
                pair_lhs.append(lhs)
            lhss[g] = pair_lhs
            ctx_p.__exit__(None, None, None)

        def emit_inv_pair(g, bp):
            mark(f"inv({g},{bp})")
            lhs = lhss[g][bp]
            bo = GSTART[g] + 2 * bp
            ov = OUT[bo:bo + 2, :, :].rearrange("b o l -> (b o) l")
            ot = st_pool.tile([128, 2048], F16, tag="ostage")
            for c in range(4):
                ps_o = pwo_pool.tile([128, 512], F32, tag="wo")
                nc.tensor.matmul(ps_o[:], lhs[:],
                                 cinv[:, c * 512:(c + 1) * 512],
                                 start=True, stop=True)
                if c % 2 == 0:
                    nc.scalar.copy(ot[:, c * 512:(c + 1) * 512], ps_o[:])
                else:
                    nc.vector.tensor_copy(ot[:, c * 512:(c + 1) * 512],
                                          ps_o[:])
                # chunked store rides the otherwise-idle SP sequencer; all
                # qk loads were emitted in the prologue so no HOL risk
                nc.sync.dma_start(ov[:, c * 512:(c + 1) * 512],
                                  ot[:, c * 512:(c + 1) * 512])

        # Act-table warmup: the single tanh+sin set loads once, off-path
        wud = tb_pool.tile([64, 1], F32, tag="dummy", name="wud")
        nc.scalar.activation(wud[:], ident[0:64, 0:1], AF.Tanh)

        # Software pipeline, 2 groups deep: group g's stages are spread
        # under DFT(g+1) (tanh/e2/wmix/xt) and DFT(g+2) (inv+out) so the
        # in-order engine queues never head-of-line-block the next group.
        for b in range(B):
            emit_qk_dma(b)

        NG = len(GROUPS)
        pre = {}
        post = {}

        def at(d, slot, fn):
            d.setdefault(slot, []).append(fn)

        for g in range(NG):
            e_end = GSTART[g] + GROUPS[g] - 1
            nx = GROUPS[g + 1] if g + 1 < NG else 2
            at(pre, e_end + 1, lambda g=g: emit_tanh(g))
            at(post, e_end + nx, lambda g=g: emit_e2(g))
            at(pre, e_end + nx + 1, lambda g=g: emit_wmix(g))
            at(post, e_end + nx + 1, lambda g=g: emit_xt(g))
            for p in range(GROUPS[g] // 2):
                at(post, e_end + nx + 2 + p, lambda g=g, p=p:
                   emit_inv_pair(g, p))

        for b in range(B):
            g = next(i for i in range(NG)
                     if GSTART[i] <= b < GSTART[i] + GROUPS[i])
            j = b - GSTART[g]
            if j == 0:
                e1s[g] = pe1_pool.tile([64, 2 * GROUPS[g] * 64], F32,
                                       tag="e1", name="e1")
            for fn in pre.get(b, []):
                fn()
            emit_dft_batch(g, j)
            for fn in post.get(b, []):
                fn()
        for s in sorted(set(list(pre) + list(post))):
            if s >= B:
                for fn in pre.get(s, []):
                    fn()
                for fn in post.get(s, []):
                    fn()

    nc.compile()
    return nc


def _host_pack(q, k, w_re, w_im):
    in_maps = []
    for h in range(H):
        qT = np.ascontiguousarray(q[:, :, h, :].transpose(0, 2, 1))
        kT = np.ascontiguousarray(k[:, :, h, :].transpose(0, 2, 1))
        qk = np.concatenate([qT, 2.0 * kT], axis=2)    # [B, L, 128]
        qkh = qk.astype(np.float16)
        qkl = (qk - qkh.astype(np.float32)).astype(np.float16)
        qkhl = np.stack([qkh.reshape(B, 128, NCH, 128),
                         qkl.reshape(B, 128, NCH, 128)], axis=2)

        wre = (w_re[h] * S_W).astype(np.float32)       # [E, E(o), M]
        wim = (w_im[h] * S_W).astype(np.float32)
        wp = np.empty((128, M, 128), dtype=np.float16)
        wp[0:64, :, 0:64] = wre.transpose(0, 2, 1)
        wp[64:128, :, 0:64] = -wim.transpose(0, 2, 1)
        wp[0:64, :, 64:128] = wim.transpose(0, 2, 1)
        wp[64:128, :, 64:128] = wre.transpose(0, 2, 1)
        in_maps.append({
            "qk": np.ascontiguousarray(qkhl),
            "wp": wp,
        })
    return in_maps


def kernel(q, k, v, w_re, w_im, _trace=False):
    q = np.asarray(q, dtype=np.float32)
    k = np.asarray(k, dtype=np.float32)
    w_re = np.asarray(w_re, dtype=np.float32)
    w_im = np.asarray(w_im, dtype=np.float32)

    if "nc" not in _CACHE:
        _CACHE["nc"] = _build()
    nc = _CACHE["nc"]

    in_maps = _host_pack(q, k, w_re, w_im)
    res = None
    for attempt in range(3):
        try:
            res = run_bass_kernel_spmd(nc, in_maps, list(range(H)),
                                       trace=_trace)
            break
        except Exception:
            if attempt == 2:
                raise
            import time as _time
            _time.sleep(5.0)
    out = np.stack([res.results[h]["out"].astype(np.float32) * S_HOST
                    for h in range(H)], axis=2)
    _CACHE["last_result"] = res
    return np.ascontiguousarray(out)  # [B, E, H, L]
